# revision 7
# baseline (speedup 1.0000x reference)
"""GQA attention kernel for 8 Trainium2 NeuronCores.

Sharding: core c = 4*b + h handles batch b (of 2) and kv-head h (of 4),
i.e. one kv head + its 4 grouped query heads. Each core computes its head
group's partial contribution to the output projection; the host sums the
4 partials per batch. No collectives.

Device math per core (S=2048, H=2048, d=128):
  QT_g = (x @ Wq_g + bq_g)^T          [d, S]   g=0..3   (fp32r matmuls)
  KT   = (x @ Wk_h)^T                 [d, S]            (bk cancels in softmax)
  V    = x @ Wv_h                     [S, d]   (computed as V^T then PE-transposed)
  S^T  = KT^T-blocks @ QT             [Sk, Sq] (bf16)
  P^T  = exp(SCALE * S^T)             (bf16, no max-subtraction: |s| <~ 6)
  y^T  = V^T-blocks.T @ P^T (PSUM accum), den = ones.T @ P^T
  yT  := y^T * (1/den broadcast)      [d, S]
  out += yT_g^T @ Wo_g                [S, H]  (fp32r, partial over this head group)
Host: out[b] = sum_h partial + (bv_rep @ Wo + bo).
"""

import numpy as np
import ml_dtypes

B = 2
S = 2048
HIDDEN = 2048
NKV = 4
GROUP = 4
D = 128
SCALE = D ** -0.5

BAND = 256            # S-columns per projection band
NBAND = S // BAND     # 8
NCH = HIDDEN // 128   # 16 contraction chunks
QTILE = 512           # queries per attention tile
NQT = S // QTILE      # 4
NSK = S // 128        # 16 key tiles

_CACHE = {}
LAST_RESULTS = None
TRACE = False
TMPDIR = None


def _build():
    import concourse.bass as bass
    import concourse.bacc as bacc
    import concourse.mybir as mybir
    import concourse.tile as tile
    from concourse.masks import make_identity

    f32 = mybir.dt.float32
    f32r = mybir.dt.float32r
    bf16 = mybir.dt.bfloat16
    EXP = mybir.ActivationFunctionType.Exp
    IDENT = mybir.ActivationFunctionType.Identity
    COPY = mybir.ActivationFunctionType.Copy

    nc = bacc.Bacc(trn_type="TRN2", target_bir_lowering=False, debug=False)

    xT = nc.dram_tensor("xT", [NBAND, 128, NCH, BAND], f32r, kind="ExternalInput").ap()
    wq = nc.dram_tensor("wq", [128, NCH, 512], f32r, kind="ExternalInput").ap()
    wk = nc.dram_tensor("wk", [128, NCH, 128], f32r, kind="ExternalInput").ap()
    wv = nc.dram_tensor("wv", [128, NCH, 128], f32r, kind="ExternalInput").ap()
    wo = nc.dram_tensor("wo", [GROUP, 128, HIDDEN], bf16, kind="ExternalInput").ap()
    bq = nc.dram_tensor("bq", [128, GROUP], f32, kind="ExternalInput").ap()
    onesk = nc.dram_tensor("onesk", [128, 1], bf16, kind="ExternalInput").ap()
    ones1 = nc.dram_tensor("ones1", [1, 128], f32, kind="ExternalInput").ap()
    out = nc.dram_tensor("out", [S, HIDDEN], f32, kind="ExternalOutput").ap()

    with tile.TileContext(nc) as tc:
        with (
            tc.tile_pool(name="const", bufs=1) as constp,
            tc.tile_pool(name="wts", bufs=1) as wtsp,
            tc.tile_pool(name="xb", bufs=2) as xbp,
            tc.tile_pool(name="qkv", bufs=1) as qkvp,
            tc.tile_pool(name="ptbuf", bufs=4) as ptp,
            tc.tile_pool(name="dens", bufs=2) as densp,
            tc.tile_pool(name="ytbuf", bufs=8) as ytp,
            tc.tile_pool(name="outbuf", bufs=2) as outp,
        ):
            # ---- constants & weights ----
            ident = constp.tile([128, 128], f32, name="ident")
            make_identity(nc, ident[:, :])
            onesk_t = constp.tile([128, 1], bf16, name="onesk_t")
            nc.sync.dma_start(out=onesk_t[:, :], in_=onesk)
            ones1_t = constp.tile([1, 128], f32, name="ones1_t")
            nc.sync.dma_start(out=ones1_t[:, :], in_=ones1)
            bq_t = constp.tile([128, GROUP], f32, name="bq_t")
            nc.sync.dma_start(out=bq_t[:, :], in_=bq)

            wq_t = wtsp.tile([128, NCH, 512], f32r, name="wq_t")
            nc.sync.dma_start(out=wq_t[:, :, :], in_=wq)
            wk_t = wtsp.tile([128, NCH, 128], f32r, name="wk_t")
            nc.sync.dma_start(out=wk_t[:, :, :], in_=wk)
            wv_t = wtsp.tile([128, NCH, 128], f32r, name="wv_t")
            nc.sync.dma_start(out=wv_t[:, :, :], in_=wv)
            wo_t = []
            for g in range(GROUP):
                t = wtsp.tile([128, HIDDEN], bf16, name=f"wo_t{g}", tag=f"wo{g}")
                nc.sync.dma_start(out=t[:, :], in_=wo[g])
                wo_t.append(t)

            # ---- persistent activations ----
            qt_t = []
            for g in range(GROUP):
                t = qkvp.tile([128, S], bf16, name=f"qt{g}", tag=f"qt{g}")
                qt_t.append(t)
            kt_t = qkvp.tile([128, S], bf16, name="kt_t")
            v_t = qkvp.tile([128, NSK * 128], bf16, name="v_t")
            vt_f = qkvp.tile([128, S], f32, name="vt_f")

            # =============== phase 1: projections ===============
            with tc.tile_pool(name="psA", bufs=1, space="PSUM") as psA:
                for b in range(NBAND):
                    band = xbp.tile([128, NCH, BAND], f32r, name="band", tag="band")
                    nc.sync.dma_start(out=band[:, :, :], in_=xT[b])
                    bsl = slice(b * BAND, (b + 1) * BAND)

                    # K^T accumulation
                    pk = psA.tile([128, BAND], f32, name="pk", tag="pacc", bufs=2)
                    for c in range(NCH):
                        nc.tensor.matmul(
                            out=pk[:, :],
                            lhsT=wk_t[:, c, :],
                            rhs=band[:, c, :],
                            start=(c == 0), stop=(c == NCH - 1),
                        )
                    nc.scalar.activation(kt_t[:, bsl], pk[:, :], COPY)

                    # V^T accumulation (f32, transposed to V per 128-block later)
                    pv = psA.tile([128, BAND], f32, name="pv", tag="pacc", bufs=2)
                    for c in range(NCH):
                        nc.tensor.matmul(
                            out=pv[:, :],
                            lhsT=wv_t[:, c, :],
                            rhs=band[:, c, :],
                            start=(c == 0), stop=(c == NCH - 1),
                        )
                    nc.scalar.activation(vt_f[:, bsl], pv[:, :], COPY)

                    # Q^T per local head
                    for g in range(GROUP):
                        pq = psA.tile([128, BAND], f32, name="pq", tag="pacc", bufs=2)
                        for c in range(NCH):
                            nc.tensor.matmul(
                                out=pq[:, :],
                                lhsT=wq_t[:, c, g * 128:(g + 1) * 128],
                                rhs=band[:, c, :],
                                start=(c == 0), stop=(c == NCH - 1),
                            )
                        nc.scalar.activation(
                            qt_t[g][:, bsl], pq[:, :], IDENT,
                            bias=bq_t[:, g:g + 1],
                        )

                    # transpose V^T band -> V (2 sk-tiles per band)
                    for t in range(BAND // 128):
                        sk = b * (BAND // 128) + t
                        pt = psA.tile([128, 128], f32, name="ptr", tag="pacc", bufs=2)
                        nc.tensor.transpose(
                            pt[:, :], vt_f[:, sk * 128:(sk + 1) * 128], ident[:, :]
                        )
                        nc.scalar.activation(
                            v_t[:, sk * 128:(sk + 1) * 128], pt[:, :], COPY
                        )

            # =============== phase 2+3: attention + out-projection ===============
            with tc.tile_pool(name="psB", bufs=1, space="PSUM") as psB:
                yt_all = {}

                def attn_qtile(qt):
                    qsl = slice(qt * QTILE, (qt + 1) * QTILE)
                    for g in range(GROUP):
                        py = psB.tile([128, QTILE], f32, name="py", tag="yacc", bufs=2)
                        pden = psB.tile([1, QTILE], f32, name="pden", tag="den", bufs=2)
                        for sk in range(NSK):
                            ksl = slice(sk * 128, (sk + 1) * 128)
                            ps = psB.tile([128, QTILE], f32, name="ps", tag="mm", bufs=4)
                            nc.tensor.matmul(
                                out=ps[:, :], lhsT=kt_t[:, ksl], rhs=qt_t[g][:, qsl],
                                start=True, stop=True,
                            )
                            ptile = ptp.tile([128, QTILE], bf16, name="ptile", tag="pt")
                            nc.scalar.activation(ptile[:, :], ps[:, :], EXP, scale=SCALE)
                            nc.tensor.matmul(
                                out=py[:, :], lhsT=v_t[:, ksl], rhs=ptile[:, :],
                                start=(sk == 0), stop=(sk == NSK - 1),
                            )
                            nc.tensor.matmul(
                                out=pden[:, :], lhsT=onesk_t[:, :], rhs=ptile[:, :],
                                start=(sk == 0), stop=(sk == NSK - 1),
                            )
                        # normalize: yT = py * (1/den broadcast over partitions)
                        recip = densp.tile([1, QTILE], f32, name="recip", tag="recip")
                        nc.vector.reciprocal(recip[:, :], pden[:, :])
                        pb = psB.tile([128, QTILE], f32, name="pb", tag="mm", bufs=4)
                        nc.tensor.matmul(
                            out=pb[:, :], lhsT=ones1_t[:, :],
                            rhs=recip[:, :], start=True, stop=True,
                        )
                        bcast = densp.tile([128, QTILE], f32, name="bcast", tag="bcast")
                        nc.scalar.activation(bcast[:, :], pb[:, :], COPY)
                        yt = ytp.tile([128, QTILE], bf16, name="yt", tag="yt")
                        nc.vector.tensor_mul(yt[:, :], py[:, :], bcast[:, :])
                        yt_all[(qt, g)] = yt

                def outproj_qtile(qt):
                    for i in range(QTILE // 128):
                        outs = outp.tile([128, HIDDEN], f32, name="outs", tag="outs")
                        pos = [
                            psB.tile([128, 512], f32, name=f"po{j}", tag="mm", bufs=4)
                            for j in range(4)
                        ]
                        for g in range(GROUP):
                            lhs = yt_all[(qt, g)][:, i * 128:(i + 1) * 128]
                            for j in range(4):
                                nc.tensor.matmul(
                                    out=pos[j][:, :], lhsT=lhs,
                                    rhs=wo_t[g][:, j * 512:(j + 1) * 512],
                                    start=(g == 0), stop=(g == GROUP - 1),
                                )
                        for j in range(4):
                            nc.vector.tensor_copy(outs[:, j * 512:(j + 1) * 512], pos[j][:, :])
                        r0 = qt * QTILE + i * 128
                        nc.sync.dma_start(out=out[r0:r0 + 128, :], in_=outs[:, :])

                for qt in range(NQT):
                    attn_qtile(qt)
                    if qt > 0:
                        outproj_qtile(qt - 1)
                outproj_qtile(NQT - 1)

    nc.finalize()
    return nc


def _get_nc():
    if "nc" not in _CACHE:
        _CACHE["nc"] = _build()
    return _CACHE["nc"]


def kernel(x, Wq, bq, Wk, bk, Wv, bv, Wo, bo):
    global LAST_RESULTS
    from concourse.bass_utils import run_bass_kernel_spmd

    x = np.asarray(x, np.float32)
    Wq = np.asarray(Wq, np.float32)
    Wk = np.asarray(Wk, np.float32)
    Wv = np.asarray(Wv, np.float32)
    Wo = np.asarray(Wo, np.float32)
    bq = np.asarray(bq, np.float32)
    bv = np.asarray(bv, np.float32)
    bo = np.asarray(bo, np.float32)

    nc = _get_nc()

    onesk_np = np.ones((128, 1), ml_dtypes.bfloat16)
    ones1_np = np.ones((1, 128), np.float32)

    in_maps = []
    for c in range(8):
        b, h = divmod(c, NKV)
        xT = x[b].T  # [HIDDEN, S]
        xTh = np.ascontiguousarray(
            xT.reshape(NCH, 128, NBAND, BAND).transpose(2, 1, 0, 3)
        )
        wqh = np.ascontiguousarray(
            Wq[:, h * 512:(h + 1) * 512].reshape(NCH, 128, 512).transpose(1, 0, 2)
        )
        wkh = np.ascontiguousarray(
            Wk[:, h * 128:(h + 1) * 128].reshape(NCH, 128, 128).transpose(1, 0, 2)
        )
        wvh = np.ascontiguousarray(
            Wv[:, h * 128:(h + 1) * 128].reshape(NCH, 128, 128).transpose(1, 0, 2)
        )
        woh = np.ascontiguousarray(
            Wo[h * 512:(h + 1) * 512, :].reshape(GROUP, 128, HIDDEN)
        ).astype(ml_dtypes.bfloat16)
        bqh = np.ascontiguousarray(
            bq[h * 512:(h + 1) * 512].reshape(GROUP, 128).T
        )
        in_maps.append({
            "xT": xTh, "wq": wqh, "wk": wkh, "wv": wvh, "wo": woh,
            "bq": bqh, "onesk": onesk_np, "ones1": ones1_np,
        })

    res = run_bass_kernel_spmd(
        nc, in_maps, list(range(8)), trace=TRACE, tmpdir=TMPDIR
    )
    LAST_RESULTS = res

    # host-side constant bias: (bv repeated per head group) @ Wo + bo
    bv_rep = np.broadcast_to(
        bv.reshape(NKV, 1, D), (NKV, GROUP, D)
    ).reshape(HIDDEN)
    bias_row = bv_rep @ Wo + bo  # [HIDDEN]

    out = np.empty((B, S, HIDDEN), np.float32)
    for b in range(B):
        acc = res.results[b * NKV + 0]["out"].astype(np.float32)
        for h in range(1, NKV):
            acc = acc + res.results[b * NKV + h]["out"]
        out[b] = acc + bias_row
    return out


# revision 8
# speedup vs baseline: 1.1881x; 1.1881x over previous
"""GQA attention kernel for 8 Trainium2 NeuronCores.

Sharding: core c = 4*b + h handles batch b (of 2) and kv-head h (of 4),
i.e. one kv head + its 4 grouped query heads. Each core computes its head
group's partial contribution to the output projection; the host sums the
4 partials per batch. No collectives.

Device math per core (S=2048, H=2048, d=128):
  QT_g = (x @ Wq_g + bq_g)^T          [d, S]   g=0..3   (fp32r matmuls)
  KT   = (x @ Wk_h)^T                 [d, S]            (bk cancels in softmax)
  V    = x @ Wv_h                     [S, d]   (computed as V^T then PE-transposed)
  S^T  = KT^T-blocks @ QT             [Sk, Sq] (bf16)
  P^T  = exp(SCALE * S^T)             (bf16, no max-subtraction: |s| <~ 6)
  y^T  = V^T-blocks.T @ P^T (PSUM accum), den = ones.T @ P^T (PE ones-matmul)
  yT  := y^T * (1/den broadcast via gpsimd partition_broadcast)
  out += yT_g^T @ Wo_g                [S, H]  (bf16, partial over this head group)
Host: out[b] = sum_h partial + (bv_rep @ Wo + bo).
"""

import numpy as np
import ml_dtypes

B = 2
S = 2048
HIDDEN = 2048
NKV = 4
GROUP = 4
D = 128
SCALE = D ** -0.5

BAND = 256            # S-columns per projection band
NBAND = S // BAND     # 8
NCH = HIDDEN // 128   # 16 contraction chunks
QTILE = 512           # queries per attention tile
NQT = S // QTILE      # 4
NSK = S // 128        # 16 key tiles

_CACHE = {}
LAST_RESULTS = None
TRACE = False
TMPDIR = None


def _build():
    import concourse.bass as bass
    import concourse.bacc as bacc
    import concourse.mybir as mybir
    import concourse.tile as tile
    from concourse.masks import make_identity

    f32 = mybir.dt.float32
    f32r = mybir.dt.float32r
    bf16 = mybir.dt.bfloat16
    EXP = mybir.ActivationFunctionType.Exp
    IDENT = mybir.ActivationFunctionType.Identity
    COPY = mybir.ActivationFunctionType.Copy

    nc = bacc.Bacc(trn_type="TRN2", target_bir_lowering=False, debug=False)

    xT = nc.dram_tensor("xT", [NBAND, 128, NCH, BAND], f32r, kind="ExternalInput").ap()
    wq = nc.dram_tensor("wq", [4, 128, NCH, 128], f32r, kind="ExternalInput").ap()
    wk = nc.dram_tensor("wk", [128, NCH, 128], f32r, kind="ExternalInput").ap()
    wv = nc.dram_tensor("wv", [128, NCH, 128], f32r, kind="ExternalInput").ap()
    wo = nc.dram_tensor("wo", [GROUP, 128, HIDDEN], bf16, kind="ExternalInput").ap()
    bq = nc.dram_tensor("bq", [128, GROUP], f32, kind="ExternalInput").ap()
    onesk = nc.dram_tensor("onesk", [128, 1], bf16, kind="ExternalInput").ap()
    out = nc.dram_tensor("out", [S, HIDDEN], f32, kind="ExternalOutput").ap()

    with tile.TileContext(nc) as tc:
        with (
            tc.tile_pool(name="const", bufs=1) as constp,
            tc.tile_pool(name="wts", bufs=1) as wtsp,
            tc.tile_pool(name="xb", bufs=2) as xbp,
            tc.tile_pool(name="qkv", bufs=1) as qkvp,
            tc.tile_pool(name="ptbuf", bufs=4) as ptp,
            tc.tile_pool(name="dens", bufs=3) as densp,
            tc.tile_pool(name="ytbuf", bufs=8) as ytp,
            tc.tile_pool(name="outbuf", bufs=2) as outp,
        ):
            # ---- DMAs in consumption order: consts, wk, band0 (in loop), wv, wq, wo ----
            onesk_t = constp.tile([128, 1], bf16, name="onesk_t")
            nc.sync.dma_start(out=onesk_t[:, :], in_=onesk)
            bq_t = constp.tile([128, GROUP], f32, name="bq_t")
            nc.sync.dma_start(out=bq_t[:, :], in_=bq)
            ident = constp.tile([128, 128], f32, name="ident")
            make_identity(nc, ident[:, :])

            wk_t = wtsp.tile([128, NCH, 128], f32r, name="wk_t")
            nc.sync.dma_start(out=wk_t[:, :, :], in_=wk)

            # band 0 load issued before the remaining weights
            bands = [None] * NBAND
            bands[0] = xbp.tile([128, NCH, BAND], f32r, name="band", tag="band")
            nc.sync.dma_start(out=bands[0][:, :, :], in_=xT[0])

            wv_t = wtsp.tile([128, NCH, 128], f32r, name="wv_t")
            nc.sync.dma_start(out=wv_t[:, :, :], in_=wv)
            wq_t = []
            for g in range(GROUP):
                t = wtsp.tile([128, NCH, 128], f32r, name=f"wq_t{g}", tag=f"wq{g}")
                nc.sync.dma_start(out=t[:, :, :], in_=wq[g])
                wq_t.append(t)

            # ---- persistent activations ----
            qt_t = []
            for g in range(GROUP):
                t = qkvp.tile([128, S], bf16, name=f"qt{g}", tag=f"qt{g}")
                qt_t.append(t)
            kt_t = qkvp.tile([128, S], bf16, name="kt_t")
            v_t = qkvp.tile([128, NSK * 128], bf16, name="v_t")
            vt_f = qkvp.tile([128, S], f32, name="vt_f")

            # =============== phase 1: projections ===============
            with tc.tile_pool(name="psA", bufs=1, space="PSUM") as psA:
                for b in range(NBAND):
                    if bands[b] is None:
                        bands[b] = xbp.tile(
                            [128, NCH, BAND], f32r, name="band", tag="band"
                        )
                        nc.sync.dma_start(out=bands[b][:, :, :], in_=xT[b])
                    band = bands[b]
                    bsl = slice(b * BAND, (b + 1) * BAND)

                    # K^T accumulation
                    pk = psA.tile([128, BAND], f32, name="pk", tag="pacc", bufs=3)
                    for c in range(NCH):
                        nc.tensor.matmul(
                            out=pk[:, :],
                            lhsT=wk_t[:, c, :],
                            rhs=band[:, c, :],
                            start=(c == 0), stop=(c == NCH - 1),
                        )
                    nc.scalar.activation(kt_t[:, bsl], pk[:, :], COPY)

                    # V^T accumulation (f32, transposed to V per 128-block later)
                    pv = psA.tile([128, BAND], f32, name="pv", tag="pacc", bufs=3)
                    for c in range(NCH):
                        nc.tensor.matmul(
                            out=pv[:, :],
                            lhsT=wv_t[:, c, :],
                            rhs=band[:, c, :],
                            start=(c == 0), stop=(c == NCH - 1),
                        )
                    nc.scalar.activation(vt_f[:, bsl], pv[:, :], COPY)

                    # Q^T per local head
                    for g in range(GROUP):
                        pq = psA.tile([128, BAND], f32, name="pq", tag="pacc", bufs=3)
                        for c in range(NCH):
                            nc.tensor.matmul(
                                out=pq[:, :],
                                lhsT=wq_t[g][:, c, :],
                                rhs=band[:, c, :],
                                start=(c == 0), stop=(c == NCH - 1),
                            )
                        nc.scalar.activation(
                            qt_t[g][:, bsl], pq[:, :], IDENT,
                            bias=bq_t[:, g:g + 1],
                        )

                    # transpose V^T band -> V (2 sk-tiles per band)
                    for t in range(BAND // 128):
                        sk = b * (BAND // 128) + t
                        pt = psA.tile([128, 128], f32, name="ptr", tag="pacc", bufs=3)
                        nc.tensor.transpose(
                            pt[:, :], vt_f[:, sk * 128:(sk + 1) * 128], ident[:, :]
                        )
                        nc.scalar.activation(
                            v_t[:, sk * 128:(sk + 1) * 128], pt[:, :], COPY
                        )

            # wo loads (needed only by out-projection, keep off the critical path)
            wo_t = []
            for g in range(GROUP):
                t = wtsp.tile([128, HIDDEN], bf16, name=f"wo_t{g}", tag=f"wo{g}")
                nc.sync.dma_start(out=t[:, :], in_=wo[g])
                wo_t.append(t)

            # =============== phase 2+3: attention + out-projection ===============
            with tc.tile_pool(name="psB", bufs=1, space="PSUM") as psB:
                yt_all = {}

                def attn_qtile(qt):
                    qsl = slice(qt * QTILE, (qt + 1) * QTILE)
                    for g in range(GROUP):
                        py = psB.tile([128, QTILE], f32, name="py", tag="yacc", bufs=2)
                        pden = psB.tile([1, QTILE], f32, name="pden", tag="den", bufs=2)
                        for skp in range(NSK // 2):
                            # paired sk tiles share one 2-bank scores tile and one exp
                            ps = psB.tile([128, 2 * QTILE], f32, name="ps", tag="sc", bufs=2)
                            for half in range(2):
                                sk = 2 * skp + half
                                nc.tensor.matmul(
                                    out=ps[:, half * QTILE:(half + 1) * QTILE],
                                    lhsT=kt_t[:, sk * 128:(sk + 1) * 128],
                                    rhs=qt_t[g][:, qsl],
                                    start=True, stop=True,
                                )
                            ptile = ptp.tile([128, 2 * QTILE], bf16, name="ptile", tag="pt")
                            nc.scalar.activation(ptile[:, :], ps[:, :], EXP, scale=SCALE)
                            for half in range(2):
                                sk = 2 * skp + half
                                hsl = slice(half * QTILE, (half + 1) * QTILE)
                                nc.tensor.matmul(
                                    out=py[:, :],
                                    lhsT=v_t[:, sk * 128:(sk + 1) * 128],
                                    rhs=ptile[:, hsl],
                                    start=(sk == 0), stop=(sk == NSK - 1),
                                )
                                nc.tensor.matmul(
                                    out=pden[:, :],
                                    lhsT=onesk_t[:, :],
                                    rhs=ptile[:, hsl],
                                    start=(sk == 0), stop=(sk == NSK - 1),
                                )
                        # normalize: yT = py * (1/den) broadcast over partitions
                        recip = densp.tile([1, QTILE], f32, name="recip", tag="recip")
                        nc.vector.reciprocal(recip[:, :], pden[:, :])
                        bcast = densp.tile([128, QTILE], f32, name="bcast", tag="bcast")
                        nc.gpsimd.partition_broadcast(bcast[:, :], recip[:, :])
                        yt = ytp.tile([128, QTILE], bf16, name="yt", tag="yt")
                        nc.vector.tensor_mul(yt[:, :], py[:, :], bcast[:, :])
                        yt_all[(qt, g)] = yt

                def outproj_qtile(qt):
                    for i in range(QTILE // 128):
                        outs = outp.tile([128, HIDDEN], f32, name="outs", tag="outs")
                        po = [
                            psB.tile([128, 2 * QTILE], f32, name=f"po{jp}", tag="sc", bufs=2)
                            for jp in range(2)
                        ]
                        for g in range(GROUP):
                            lhs = yt_all[(qt, g)][:, i * 128:(i + 1) * 128]
                            for j in range(4):
                                nc.tensor.matmul(
                                    out=po[j // 2][:, (j % 2) * 512:(j % 2 + 1) * 512],
                                    lhsT=lhs,
                                    rhs=wo_t[g][:, j * 512:(j + 1) * 512],
                                    start=(g == 0), stop=(g == GROUP - 1),
                                )
                        for jp in range(2):
                            nc.vector.tensor_copy(
                                outs[:, jp * 1024:(jp + 1) * 1024], po[jp][:, :]
                            )
                        r0 = qt * QTILE + i * 128
                        nc.sync.dma_start(out=out[r0:r0 + 128, :], in_=outs[:, :])

                for qt in range(NQT):
                    attn_qtile(qt)
                    outproj_qtile(qt)

    nc.finalize()
    return nc


def _get_nc():
    if "nc" not in _CACHE:
        _CACHE["nc"] = _build()
    return _CACHE["nc"]


def kernel(x, Wq, bq, Wk, bk, Wv, bv, Wo, bo):
    global LAST_RESULTS
    from concourse.bass_utils import run_bass_kernel_spmd

    x = np.asarray(x, np.float32)
    Wq = np.asarray(Wq, np.float32)
    Wk = np.asarray(Wk, np.float32)
    Wv = np.asarray(Wv, np.float32)
    Wo = np.asarray(Wo, np.float32)
    bq = np.asarray(bq, np.float32)
    bv = np.asarray(bv, np.float32)
    bo = np.asarray(bo, np.float32)

    nc = _get_nc()

    onesk_np = np.ones((128, 1), ml_dtypes.bfloat16)

    in_maps = []
    for c in range(8):
        b, h = divmod(c, NKV)
        xT = x[b].T  # [HIDDEN, S]
        xTh = np.ascontiguousarray(
            xT.reshape(NCH, 128, NBAND, BAND).transpose(2, 1, 0, 3)
        )
        # wq[g]: [128, NCH, 128] per local head
        wqh = np.ascontiguousarray(
            Wq[:, h * 512:(h + 1) * 512]
            .reshape(NCH, 128, GROUP, 128).transpose(2, 1, 0, 3)
        )
        wkh = np.ascontiguousarray(
            Wk[:, h * 128:(h + 1) * 128].reshape(NCH, 128, 128).transpose(1, 0, 2)
        )
        wvh = np.ascontiguousarray(
            Wv[:, h * 128:(h + 1) * 128].reshape(NCH, 128, 128).transpose(1, 0, 2)
        )
        woh = np.ascontiguousarray(
            Wo[h * 512:(h + 1) * 512, :].reshape(GROUP, 128, HIDDEN)
        ).astype(ml_dtypes.bfloat16)
        bqh = np.ascontiguousarray(
            bq[h * 512:(h + 1) * 512].reshape(GROUP, 128).T
        )
        in_maps.append({
            "xT": xTh, "wq": wqh, "wk": wkh, "wv": wvh, "wo": woh,
            "bq": bqh, "onesk": onesk_np,
        })

    res = run_bass_kernel_spmd(
        nc, in_maps, list(range(8)), trace=TRACE, tmpdir=TMPDIR
    )
    LAST_RESULTS = res

    # host-side constant bias: (bv repeated per head group) @ Wo + bo
    bv_rep = np.broadcast_to(
        bv.reshape(NKV, 1, D), (NKV, GROUP, D)
    ).reshape(HIDDEN)
    bias_row = bv_rep @ Wo + bo  # [HIDDEN]

    out = np.empty((B, S, HIDDEN), np.float32)
    for b in range(B):
        acc = res.results[b * NKV + 0]["out"].astype(np.float32)
        for h in range(1, NKV):
            acc = acc + res.results[b * NKV + h]["out"]
        out[b] = acc + bias_row
    return out
